# revision 22
# baseline (speedup 1.0000x reference)
"""Multi-head self-attention on 8 Trainium2 NeuronCores.

Strategy (batch x head-group sharding):
  - 2 batches x 4 head-groups -> each core owns batch b = core//4 and
    heads 4g..4g+3 (g = core%4): a 256-column slice of Wq/Wk/Wv and the
    matching 256-row slice of Wo, applied to one batch's tokens.
  - All matmul operands are bf16 (cast on the host): full PE rate and
    half the SBUF/DMA traffic of fp32.
  - Q/K/V are projected in [e, s] layout (weights stationary); V is
    then flipped to [s, e] via the DMA xbar transpose engine (one
    transpose per (head, half) into a dense tile, then strided DVE
    copies into the ones-augmented AV layout [V_h | 1]), so the PE
    never transposes anything.
  - Scores are computed transposed, ST[k, q] = K^T Q, two heads
    row-packed into the PE array (64-wide contraction per head).
  - softmax exp: most k-tiles on ACT (table exp, bf16 out); 3 of every
    16 on the DVE via the Schraudolph bit trick (affine fp32->int32 in
    one tensor_scalar, then an f32r rounding copy), which keeps the
    ACT exp stream -- the critical path -- 19% shorter. The softmax
    denominator rides the AV matmul via the ones column.
  - Per-head normalization happens on the transposed attention matrix
    right before the output projection; partial outputs are written
    bf16 and the 4 per-batch partials are summed on host (the Wo
    row-parallel all-reduce) with bo added there.

Schedule (engines execute in emission order, so placement == schedule):
  x is DMAd in 4 big chunks split across the two HWDGE queues (sync +
  activation) with wk first; the lead-in projects only K sl0 / Q sl0
  for the first 1024 tokens, then attention pair 0 streams ACT-bound
  while every other projection tile is emitted (in half-tile items)
  into specific kt slots of its blocks. AV matmuls trail the exp
  stream by 5-8 k-tiles (catching up over the last slots) so a block's
  first AV never waits on the previous block's tail chain (in-order
  engines: a blocked instruction stalls everything behind it). Pair 1
  carries the output projections of each q-chunk in the next block.
PSUM: scores 2x[128,1024] (4 banks) + AV accumulators 2x[65,512]
  (2 banks) + single-buffered proj/outproj [128,1024] (2 banks) = 8.
"""
import sys

sys.path.insert(0, "/opt/trn_rl_repo")

import numpy as np
import ml_dtypes

import concourse.bacc as bacc
import concourse.tile as tile
from concourse import mybir
from concourse.bass_utils import run_bass_kernel_spmd

AF = mybir.ActivationFunctionType
F32 = mybir.dt.float32
F32R = mybir.dt.float32r
I32 = mybir.dt.int32
BF = mybir.dt.bfloat16
BF_NP = ml_dtypes.bfloat16

N_CORES = 8
D = 1024          # model dim
S = 2048          # tokens per core (one batch)
E = 256           # per-core projection width (4 heads x 64)
HD = 64           # head dim
P = 128           # partitions
QC = 512          # q-chunk
SC = 1024         # projection s-chunk
DC = D // P       # 8
N_KT = S // P     # 16
N_QC = S // QC    # 4
N_SC = S // SC    # 2
EW = HD + 1       # per-head V width with ones column

SCHR_KT = {0: (), 1: (5, 11)}   # per-pair k-tiles with DVE exp (Schraudolph)
SCHR_A = float(2**23 / np.log(2.0)) / 8.0          # folds the 1/8 scale
SCHR_B = float(127 * 2**23 - 0.043677448 * 2**23 + 0.5)


def build_attention_core(with_qkv_bias=False):
    scale = 1.0 / np.sqrt(np.float32(HD))

    nc = bacc.Bacc("TRN2", target_bir_lowering=False)
    xT = nc.dram_tensor("xT", [P, DC, S], BF, kind="ExternalInput")
    wq = nc.dram_tensor("wq", [P, DC, E], BF, kind="ExternalInput")
    wk = nc.dram_tensor("wk", [P, DC, E], BF, kind="ExternalInput")
    wv = nc.dram_tensor("wv", [P, DC, E], BF, kind="ExternalInput")
    wo = nc.dram_tensor("wo", [P, 2, D], BF, kind="ExternalInput")
    bq = nc.dram_tensor("bq", [P, 2], F32, kind="ExternalInput")
    bk = nc.dram_tensor("bk", [P, 2], F32, kind="ExternalInput")
    bv = nc.dram_tensor("bv", [P, 2], F32, kind="ExternalInput")
    out = nc.dram_tensor("out", [S, D], BF, kind="ExternalOutput")

    with tile.TileContext(nc) as tc:
        with (
            tc.tile_pool(name="persist", bufs=1) as persist,
            tc.tile_pool(name="attp", bufs=6) as attp,
            tc.tile_pool(name="upool", bufs=8) as upool,
            tc.tile_pool(name="u32p", bufs=1) as u32p,
            tc.tile_pool(name="urp", bufs=3) as urp,
            tc.tile_pool(name="vtrp", bufs=2) as vtrp,
            tc.tile_pool(name="small", bufs=2) as small,
            tc.tile_pool(name="outp", bufs=2) as outp,
            tc.tile_pool(name="psS", bufs=2, space="PSUM") as psS,
            tc.tile_pool(name="psP", bufs=2, space="PSUM") as psP,
            tc.tile_pool(name="psQ", bufs=1, space="PSUM") as psQ,
        ):
            # ---- DMAs: wk first, x in 4 big chunks on both queues --------
            w_sb = {}
            for nm in ("k", "v", "q"):
                w_sb[nm] = persist.tile([P, DC, E], BF, tag=f"w_{nm}",
                                        name=f"w_{nm}")
            wo_sb = persist.tile([P, 2, D], BF)
            x_sb = persist.tile([P, DC, S], BF)

            nc.sync.dma_start(w_sb["k"][:], wk[:])
            H = DC // 2
            for sc in range(N_SC):
                for o in range(H):
                    nc.sync.dma_start(x_sb[:, o, sc * SC:(sc + 1) * SC],
                                      xT[:, o, sc * SC:(sc + 1) * SC])
                for o in range(H, DC):
                    nc.scalar.dma_start(x_sb[:, o, sc * SC:(sc + 1) * SC],
                                        xT[:, o, sc * SC:(sc + 1) * SC])
            nc.scalar.dma_start(w_sb["q"][:], wq[:])
            nc.scalar.dma_start(w_sb["v"][:], wv[:])
            nc.scalar.dma_start(wo_sb[:], wo[:])

            # HAM warm-up: ~4us of dependency-free dummy matmuls so the
            # real projections run at 2.4 GHz the moment x lands.
            wu_w = persist.tile([P, P], BF, tag="wu")
            nc.gpsimd.memset(wu_w[:], 0.0)
            wu_ps = psQ.tile([P, SC], F32, tag="Q", name="wu_ps")
            for _ in range(40):
                nc.tensor.matmul(wu_ps[:, 0:P], wu_w[:], wu_w[:],
                                 start=True, stop=True)

            bias_t = {}
            if with_qkv_bias:
                for nm, t in (("q", bq), ("k", bk), ("v", bv)):
                    bt = persist.tile([P, 2], F32, tag=f"b_{nm}")
                    nc.sync.dma_start(bt[:], t[:])
                    bias_t[nm] = bt

            # ---- persistent activations ----------------------------------
            KT = persist.tile([P, 2, S], BF, tag="KT")   # [e, slice, s]
            QT = persist.tile([P, 2, S], BF, tag="QT")
            VT = persist.tile([P, 2, S], BF, tag="VT")
            # AV stationary: per k-chunk [V_h0|1|V_h1|1|V_h2|1|V_h3|1]
            V_sb = persist.tile([P, N_KT, 4 * EW], BF, tag="V")
            V_r = V_sb[:].rearrange("p c (h u) -> p c h u", u=EW)
            V32 = persist.tile([P, N_KT, 4 * EW], F32R, tag="V32")
            V32_r = V32[:].rearrange("p c (h u) -> p c h u", u=EW)
            V32f_r = V32[:].bitcast(F32).rearrange("p c (h u) -> p c h u", u=EW)
            for h in range(4):
                nc.gpsimd.memset(V_r[:, :, h, HD], 1.0)
                if h >= 2:
                    nc.gpsimd.memset(V32f_r[:, :, h, HD], 1.0)

            # ---- projection emitters (two-half items) --------------------
            dsts = {"k": KT, "q": QT, "v": VT}

            def proj_h1(nm, sl, sc):
                s0 = sc * SC
                ps = psQ.tile([P, SC], F32, tag="Q", name=f"ps_{nm}")
                for o in range(DC // 2):
                    for hh in range(SC // 512):
                        nc.tensor.matmul(
                            ps[:, hh * 512:(hh + 1) * 512],
                            w_sb[nm][:, o, sl * P:(sl + 1) * P],
                            x_sb[:, o, s0 + hh * 512:s0 + (hh + 1) * 512],
                            start=(o == 0), stop=False,
                        )
                return ps

            def proj_h2(nm, sl, sc, ps):
                s0 = sc * SC
                for o in range(DC // 2, DC):
                    for hh in range(SC // 512):
                        nc.tensor.matmul(
                            ps[:, hh * 512:(hh + 1) * 512],
                            w_sb[nm][:, o, sl * P:(sl + 1) * P],
                            x_sb[:, o, s0 + hh * 512:s0 + (hh + 1) * 512],
                            start=False, stop=(o == DC - 1),
                        )
                dst = dsts[nm][:, sl, s0:s0 + SC]
                if with_qkv_bias:
                    nc.vector.tensor_tensor(
                        dst, ps[:],
                        bias_t[nm][:, sl:sl + 1].to_broadcast((P, SC)),
                        mybir.AluOpType.add)
                else:
                    nc.vector.tensor_copy(dst, ps[:])

            def emit_proj(nm, sl, sc):
                proj_h2(nm, sl, sc, proj_h1(nm, sl, sc))

            def emit_vflip(h, sc):
                """Transpose head h's V tokens [sc*SC,(sc+1)*SC) into V_sb."""
                sl, h2 = divmod(h, 2)
                vtr = vtrp.tile([P, SC // P, HD], BF, tag="vtr")
                nc.sync.dma_start_transpose(
                    vtr[:],
                    VT[h2 * HD:(h2 + 1) * HD, sl, sc * SC:(sc + 1) * SC])
                c0 = sc * (SC // P)
                nc.vector.tensor_copy(
                    V_r[:, c0:c0 + SC // P, h, 0:HD], vtr[:])
                if h >= 2:
                    nc.vector.tensor_copy(
                        V32_r[:, c0:c0 + SC // P, h, 0:HD], vtr[:])

            # ---- attention -----------------------------------------------
            def emit_block(p, qc, sched, lag=5):
                """Scores+exp+AV for head pair p, q-chunk qc.

                sched: {kt: [callables]} -- projection/outproj work emitted
                into that kt slot. AV trails exp by `lag` k-tiles, catching
                up over the last `lag` slots.
                """
                q0 = qc * QC
                pa = [psP.tile([EW, QC], F32, tag="P", name=f"pa{h}")
                      for h in range(2)]

                def emit_av(kt, ut, f32r):
                    vsrc = V32 if f32r else V_sb
                    for h in range(2):
                        nc.tensor.matmul(
                            pa[h][:],
                            vsrc[:, kt, (2 * p + h) * EW:(2 * p + h + 1) * EW],
                            ut[:, h * QC:(h + 1) * QC],
                            start=(kt == 0), stop=(kt == N_KT - 1))

                pend = []
                for kt in range(N_KT):
                    k0 = kt * P
                    st = psS.tile([P, 2 * QC], F32, tag="S", name="st")
                    nc.tensor.matmul(
                        st[:, 0:QC],
                        KT[0:HD, p, k0:k0 + P], QT[0:HD, p, q0:q0 + QC],
                        tile_position=(0, 0), start=True, stop=True)
                    nc.tensor.matmul(
                        st[:, QC:2 * QC],
                        KT[HD:P, p, k0:k0 + P], QT[HD:P, p, q0:q0 + QC],
                        tile_position=(64, 0), start=True, stop=True)
                    if kt in SCHR_KT[p]:
                        u32 = u32p.tile([P, 2 * QC], I32, tag="U32")
                        nc.vector.tensor_scalar(
                            u32[:], st[:], SCHR_A, SCHR_B,
                            mybir.AluOpType.mult, mybir.AluOpType.add)
                        ur = urp.tile([P, 2 * QC], F32R, tag="UR")
                        nc.vector.tensor_copy(ur[:], u32[:].bitcast(F32))
                        pend.append((kt, ur, True))
                    else:
                        ut = upool.tile([P, 2 * QC], BF, tag="U")
                        nc.scalar.activation(ut[:], st[:], AF.Exp,
                                             scale=float(scale))
                        pend.append((kt, ut, False))
                    hi = kt - lag + max(0, kt - 8)
                    while pend and pend[0][0] <= hi:
                        emit_av(*pend.pop(0))
                    for fn in sched.get(kt, ()):
                        fn()
                for item in pend:
                    emit_av(*item)
                return pa

            def emit_tail(p, qc, pa):
                """Normalize pair p's attention -> attnT (bf16, persists)."""
                rsb = small.tile([1, 2 * QC], F32, tag="rsb")
                for h in range(2):
                    nc.vector.tensor_copy(
                        rsb[0:1, h * QC:(h + 1) * QC], pa[h][HD:EW, :])
                rinv1 = small.tile([1, 2 * QC], F32, tag="rinv1")
                nc.vector.reciprocal_approx_fast(rinv1[:], rsb[:])
                rb = small.tile([HD, 2 * QC], F32, tag="rb")
                nc.gpsimd.partition_broadcast(rb[:], rinv1[0:1, :])
                attnT = attp.tile([P, QC], BF, tag=f"attnT_{p}_{qc}")
                for h in range(2):
                    nc.vector.tensor_tensor(
                        attnT[h * HD:(h + 1) * HD, :],
                        pa[h][0:HD, :], rb[:, h * QC:(h + 1) * QC],
                        mybir.AluOpType.mult)
                return attnT

            def emit_outproj_ss(qc, ss, attnT_by_p):
                q0 = qc * QC
                po = psQ.tile([P, D], F32, tag="Q", name="po")
                for p in range(2):
                    for oc in range(D // 512):
                        nc.tensor.matmul(
                            po[:, oc * 512:(oc + 1) * 512],
                            attnT_by_p[p][:, ss * P:(ss + 1) * P],
                            wo_sb[:, p, oc * 512:(oc + 1) * 512],
                            start=(p == 0), stop=(p == 1))
                for oc in range(D // 512):
                    osb = outp.tile([P, 512], BF, tag="osb2", name="osb")
                    nc.vector.tensor_copy(
                        osb[:], po[:, oc * 512:(oc + 1) * 512])
                    nc.sync.dma_start(
                        out[q0 + ss * P:q0 + (ss + 1) * P,
                            oc * 512:(oc + 1) * 512], osb[:])

            def emit_outproj_final(qc, attnT_by_p):
                # Tail outproj: [128,512] units pipelined 2-deep through the
                # pa slots (free once the tail normalize has consumed them).
                q0 = qc * QC
                for ss in range(QC // P):
                    for oc in range(D // 512):
                        po = psP.tile([P, 512], F32, tag="P", name="po2")
                        for p in range(2):
                            nc.tensor.matmul(
                                po[:],
                                attnT_by_p[p][:, ss * P:(ss + 1) * P],
                                wo_sb[:, p, oc * 512:(oc + 1) * 512],
                                start=(p == 0), stop=(p == 1))
                        osb = outp.tile([P, 512], BF, tag="osb2")
                        nc.vector.tensor_copy(osb[:], po[:])
                        nc.sync.dma_start(
                            out[q0 + ss * P:q0 + (ss + 1) * P,
                                oc * 512:(oc + 1) * 512], osb[:])

            # ---- schedule ------------------------------------------------
            emit_proj("k", 0, 0)
            emit_proj("q", 0, 0)

            ctx = {}

            def H1(nm, sl, sc):
                return lambda: ctx.__setitem__(
                    (nm, sl, sc), proj_h1(nm, sl, sc))

            def H2(nm, sl, sc):
                return lambda: proj_h2(nm, sl, sc, ctx.pop((nm, sl, sc)))

            F = lambda h, sc: (lambda: emit_vflip(h, sc))
            p0_sched = [
                {0: [H1("k", 0, 1)], 1: [H2("k", 0, 1)],
                 2: [H1("v", 0, 0)], 3: [H2("v", 0, 0)],
                 4: [F(0, 0)], 5: [F(1, 0)],
                 6: [H1("v", 0, 1)], 7: [H2("v", 0, 1)],
                 8: [F(0, 1)], 9: [F(1, 1)]},                   # qc0
                {0: [H1("q", 0, 1)], 2: [H2("q", 0, 1)],
                 4: [H1("k", 1, 0)], 6: [H2("k", 1, 0)],
                 8: [H1("v", 1, 0)], 10: [H2("v", 1, 0)],
                 12: [F(2, 0)], 13: [F(3, 0)]},                 # qc1
                {},                                             # qc2
                {0: [H1("q", 1, 0)], 2: [H2("q", 1, 0)]},       # qc3
            ]
            p1_extra = [
                {0: [H1("k", 1, 1)], 2: [H2("k", 1, 1)],
                 4: [H1("v", 1, 1)], 6: [H2("v", 1, 1)],
                 8: [F(2, 1)], 10: [F(3, 1)]},                  # p1 qc0
                {0: [H1("q", 1, 1)], 2: [H2("q", 1, 1)]},       # p1 qc1
                {}, {},
            ]

            attnT0 = []
            for qc in range(N_QC):
                pa = emit_block(0, qc, p0_sched[qc], lag=(8 if qc == 0 else 5))
                attnT0.append(emit_tail(0, qc, pa))

            attnT1_prev = None
            for qc in range(N_QC):
                sched = dict(p1_extra[qc])
                if attnT1_prev is not None:
                    pair = (attnT0[qc - 1], attnT1_prev)
                    for ss in range(QC // P):
                        sched.setdefault(4 + 3 * ss, []).append(
                            (lambda ss=ss, pair=pair, q=qc - 1:
                             emit_outproj_ss(q, ss, pair)))
                pa = emit_block(1, qc, sched, lag=5)
                attnT1_prev = emit_tail(1, qc, pa)
            emit_outproj_final(N_QC - 1, (attnT0[-1], attnT1_prev))

    nc.compile()
    return nc


_NC_CACHE = {}


def _get_nc(with_qkv_bias):
    key = with_qkv_bias
    if key not in _NC_CACHE:
        _NC_CACHE[key] = build_attention_core(with_qkv_bias)
    return _NC_CACHE[key]


def _pack_pdm(a):
    """[D, M] -> [128, D//128, M] partition-major, bf16."""
    Dd, M = a.shape
    return np.ascontiguousarray(
        a.reshape(Dd // P, P, M).transpose(1, 0, 2).astype(BF_NP))


def run_attention(x, Wq, bq, Wk, bk, Wv, bv, Wo, bo, trace=False):
    B, S_, D_ = x.shape
    assert (B, S_, D_) == (2, S, D)
    with_qkv_bias = bool(np.any(bq) or np.any(bk) or np.any(bv))
    nc = _get_nc(with_qkv_bias)
    in_maps = []
    for c in range(N_CORES):
        b, g = divmod(c, N_CORES // 2)
        sl = slice(g * E, (g + 1) * E)
        xTb = np.ascontiguousarray(x[b].T)  # [D, S]
        in_maps.append({
            "xT": _pack_pdm(xTb),
            "wq": _pack_pdm(Wq[:, sl]),
            "wk": _pack_pdm(Wk[:, sl]),
            "wv": _pack_pdm(Wv[:, sl]),
            "wo": np.ascontiguousarray(
                Wo[sl, :].reshape(2, P, D).transpose(1, 0, 2)
                .astype(BF_NP)),
            "bq": np.ascontiguousarray(
                bq[sl].reshape(2, P).T.astype(np.float32)),
            "bk": np.ascontiguousarray(
                bk[sl].reshape(2, P).T.astype(np.float32)),
            "bv": np.ascontiguousarray(
                bv[sl].reshape(2, P).T.astype(np.float32)),
        })
    res = run_bass_kernel_spmd(nc, in_maps, core_ids=list(range(N_CORES)),
                               trace=trace)
    outs = []
    for b in range(2):
        acc = np.zeros((S, D), dtype=np.float32)
        for g in range(N_CORES // 2):
            acc += np.asarray(res.results[b * 4 + g]["out"]).astype(np.float32)
        outs.append(acc + np.asarray(bo, dtype=np.float32)[None, :])
    return np.stack(outs).reshape(B, S, D), res


def kernel(x, Wq, bq, Wk, bk, Wv, bv, Wo, bo):
    out, _ = run_attention(np.asarray(x), np.asarray(Wq), np.asarray(bq),
                           np.asarray(Wk), np.asarray(bk), np.asarray(Wv),
                           np.asarray(bv), np.asarray(Wo), np.asarray(bo))
    return out
